# revision 2
# baseline (speedup 1.0000x reference)
"""Trainium2 Bass kernel for masked pairwise-sigmoid GNN message passing.

Reference computation (per graph g with nodes i,j in [0,nv)):
    c = z @ Wc.T + bc ; y = z @ Wy.T + by          # [G, nv, H]
    s[g,i,j,:] = sigmoid(c[g,i,:] + y[g,j,:] + (m_i + m_j)*L - 2L)
    out[g,i,:] = sum_j s[g,i,j,:] / sum_j m[g,j]

Exact identity: with m in {0,1}, any pair with m_i==0 or m_j==0 has mask
term <= -1e10, so sigmoid underflows to exactly 0 in fp32.  Only active
nodes (m==1) contribute; for active pairs the mask term is 0.  The host
gathers active nodes per graph, the device computes the dense active x
active interaction, and the host scatters rows back (and applies the
1/n_active scale during the scatter).

Sharding: graphs sorted by active count, dealt round-robin to 8 cores in
4 slots; slot s padded to a global j-extent P_s (multiple of 8, so both
packed halving adds stay in the DVE 2x mode) and an exact i-extent PI_s,
so one SPMD program serves all cores.  Padding columns get a -1e5
additive mask (sigmoid -> 0); padding rows are discarded on scatter.

Device design (v2 — reworked from the traced baseline):
- 3 input dma_starts, one per DMA queue (sync: zT, scalar: wcT,
  gpsimd: wyT) plus one tiny single-row aux DMA (gpsimd, 2nd) carrying
  [em | ones | bc+by].  Each dma_start costs ~640ns of issue time on its
  queue, so fewer+parallel is what matters, not transfer size.
- bc+by and the pad mask are folded into the y-projection PSUM as two
  rank-1 matmuls (ones (x) em  and  bsum (x) ones), so the PSUM
  evacuations are plain table-free Copy ops.  A dummy sigmoid on a const
  AP right after the scalar queue's dma issue pins the single
  sigmoid_and_others table load (contains copy+identity too) off the
  critical path; the baseline paid 2 table loads, one mid-kernel.
- Evacuations are per (slot, ob): the first pass-1 DVE add starts as
  soon as slot0/ob0's c and y columns are out, ~1us earlier than with
  whole-row evacuations.  c is evacuated as duplicated bf16 pairs
  [c_i|c_i] (step-1 pairs enable the 2x_1P packed DVE mode for the
  broadcast add); y as plain bf16 rows.
- Reduction per slot: sigmoid (ACT, (224+FD)/1.2GHz) then a depth-2
  bf16 halving-add tree on DVE (each at 2 elem/cycle) and a final
  tensor_reduce at width P/4 (tensor_reduce never packs; halving twice
  cuts its 1x cycles in half vs the baseline's single halve).
- GPSIMD issues DMAs only (a running gpsimd tensor op blocks
  concurrently issued 2-port DVE ops on the shared POOL SBUF port).
"""

import numpy as np

import concourse.bass as bass
import concourse.mybir as mybir
import concourse.tile as tile
from concourse import bacc
from concourse.bass_utils import run_bass_kernel_spmd

F32 = mybir.dt.float32
BF16 = mybir.dt.bfloat16
N_CORES = 8
PAD_NEG = -1.0e5  # additive mask for padding columns; sigmoid(-1e5) == 0

# test.py reads this for profiling info after a traced run
_last_results = None
_program_cache = {}


def _ap(view, free_dims):
    """AP anchored at `view`'s base with custom free dims (stride, num)."""
    return bass.AP(
        tensor=view.tensor,
        offset=view.offset,
        ap=[list(view.ap[0])] + [[int(s), int(n)] for s, n in free_dims],
    )


def _build_program(P_list, PI_list, H):
    """P_list: per-slot j-extent (mult of 8); PI_list: per-slot i-extent."""
    NTOT = sum(P_list)
    ONB = max(128, NTOT)  # ones block length (lhsT slice needs >=128)
    assert H == 256
    nc = bacc.Bacc(None, target_bir_lowering=False)

    zT = nc.dram_tensor("zT", [128, 2 * NTOT], BF16, kind="ExternalInput")
    # weight chunk layout: [128, (kb0ob0 | kb1ob0 | kb0ob1 | kb1ob1) * 128]
    wcT = nc.dram_tensor("wcT", [128, 2 * H], BF16, kind="ExternalInput")
    wyT = nc.dram_tensor("wyT", [128, 2 * H], BF16, kind="ExternalInput")
    # single row: [em (NTOT) | ones (ONB) | bc+by (256)]
    aux = nc.dram_tensor("aux", [1, NTOT + ONB + 256], BF16, kind="ExternalInput")
    out = nc.dram_tensor("out", [128, 2 * NTOT], F32, kind="ExternalOutput")

    AT = mybir.ActivationFunctionType
    OP = mybir.AluOpType

    offs = [0]
    for P in P_list[:-1]:
        offs.append(offs[-1] + P)
    nslots = len(P_list)

    with tile.TileContext(nc) as tc:
        with (
            tc.tile_pool(name="singles", bufs=1) as singles,
            tc.tile_pool(name="ptp", bufs=2) as ptp,
            tc.tile_pool(name="stp", bufs=2) as stp,
            tc.tile_pool(name="hvp", bufs=2) as hvp,
            tc.tile_pool(name="hqp", bufs=2) as hqp,
            tc.tile_pool(name="oup", bufs=2) as oup,
            tc.tile_pool(name="psum", bufs=1, space="PSUM") as psum,
        ):
            z_sb = singles.tile([128, 2 * NTOT], BF16, tag="z", name="z_sb")
            w_sb = {
                "c": singles.tile([128, 2 * H], BF16, tag="wc", name="wc"),
                "y": singles.tile([128, 2 * H], BF16, tag="wy", name="wy"),
            }
            aux_sb = singles.tile([1, NTOT + ONB + 256], BF16, tag="aux", name="aux_sb")
            scratch = singles.tile([1, 1], F32, tag="scr", name="scratch")

            # one input dma_start per queue; aux rides gpsimd second (tiny)
            nc.sync.dma_start(out=z_sb[:], in_=zT[:])
            nc.scalar.dma_start(out=w_sb["c"][:], in_=wcT[:])
            nc.gpsimd.dma_start(out=w_sb["y"][:], in_=wyT[:])
            nc.gpsimd.dma_start(out=aux_sb[:], in_=aux[:])
            # dummy sigmoid: forces the single table load right here (during
            # the DMA window), so evac Copies and real sigmoids never wait
            nc.scalar.activation(
                out=scratch[:], in_=nc.const_aps.tensor(0.0, (1, 1)),
                func=AT.Sigmoid,
            )

            em_row = aux_sb[0:1, 0:NTOT]
            ones_m = aux_sb[0:1, NTOT:NTOT + 128]          # lhsT for em rank-1
            ones_n = aux_sb[0:1, NTOT:NTOT + NTOT]         # rhs for bias rank-1

            # ---- projections -> PSUM (biases+mask folded in as rank-1 terms)
            ps_t = {}
            for ob in range(2):
                for wname in ("c", "y"):
                    ps = psum.tile(
                        [128, NTOT], F32, tag=f"ps{wname}{ob}", name=f"ps{wname}{ob}"
                    )
                    for kb in range(2):
                        o0 = (2 * ob + kb) * 128
                        nc.tensor.matmul(
                            ps[:],
                            lhsT=w_sb[wname][:, o0:o0 + 128],
                            rhs=z_sb[:, kb * NTOT:(kb + 1) * NTOT],
                            start=(kb == 0),
                            stop=(kb == 1 and wname == "c"),
                        )
                    if wname == "y":
                        # pad-mask: ones_h (x) em
                        nc.tensor.matmul(
                            ps[:], lhsT=ones_m, rhs=em_row, start=False, stop=False,
                        )
                        # bias: (bc+by)_h (x) ones
                        bs = aux_sb[0:1, NTOT + ONB + 128 * ob:NTOT + ONB + 128 * (ob + 1)]
                        nc.tensor.matmul(
                            ps[:], lhsT=bs, rhs=ones_n, start=False, stop=True,
                        )
                    ps_t[wname, ob] = ps

            # ---- per (slot, ob) Copy evacuations; c' as duplicated bf16
            # pairs [c|c] (enables the 2x packed DVE add), y' as plain rows
            c2 = singles.tile([128, 4 * NTOT], BF16, tag="c2", name="c2")
            yb = singles.tile([128, 2 * NTOT], BF16, tag="yb", name="yb")
            for si, (P, PI) in enumerate(zip(P_list, PI_list)):
                col = offs[si]
                for ob in range(2):
                    cb = 2 * ob * NTOT + 2 * col
                    nc.scalar.copy(
                        out=_ap(c2[:, cb:cb + 2], [(2, PI), (1, 2)]),
                        in_=_ap(ps_t["c", ob][:, col:col + PI], [(1, PI), (0, 2)]),
                    )
                    nc.scalar.copy(
                        out=yb[:, ob * NTOT + col:ob * NTOT + col + P],
                        in_=ps_t["y", ob][:, col:col + P],
                    )

            # ---- pass 1: packed broadcast adds (slot-major)
            pts = []
            for si, (P, PI) in enumerate(zip(P_list, PI_list)):
                col = offs[si]
                pt = ptp.tile(
                    [128, 2, PI, P], BF16, tag=f"pair{si}", name=f"pair{si}"
                )
                for ob in range(2):
                    cb = 2 * ob * NTOT + 2 * col
                    in0 = _ap(c2[:, cb:cb + 2], [(2, PI), (0, P // 2), (1, 2)])
                    in1 = _ap(yb[:, ob * NTOT + col:ob * NTOT + col + P],
                              [(0, PI), (1, P)])
                    nc.vector.tensor_tensor(
                        out=pt[:, ob:ob + 1], in0=in0, in1=in1, op=OP.add
                    )
                pts.append(pt)

            # ---- pass 2: sigmoid -> halve -> halve -> reduce -> store
            for si, (P, PI) in enumerate(zip(P_list, PI_list)):
                col = offs[si]
                pt = pts[si]
                st = stp.tile([128, 2, PI, P], BF16, tag="sig", name="sig_t")
                nc.scalar.activation(out=st[:], in_=pt[:], func=AT.Sigmoid)
                hw = P // 2
                hv = hvp.tile([128, 2, PI, hw], BF16, tag="hv", name="hv_t")
                nc.vector.tensor_tensor(
                    out=hv[:], in0=st[:, :, :, 0:hw], in1=st[:, :, :, hw:P],
                    op=OP.add,
                )
                hq = hw // 2
                h2 = hqp.tile([128, 2, PI, hq], BF16, tag="hq", name="hq_t")
                nc.vector.tensor_tensor(
                    out=h2[:], in0=hv[:, :, :, 0:hq], in1=hv[:, :, :, hq:hw],
                    op=OP.add,
                )
                red = oup.tile([128, 2, PI], F32, tag="red", name="red_t")
                nc.vector.reduce_sum(out=red[:], in_=h2[:], axis=mybir.AxisListType.X)
                nc.sync.dma_start(
                    out=_ap(out[0:128, col:col + PI], [(NTOT, 2), (1, PI)]),
                    in_=red[:],
                )

    nc.finalize()
    return nc


def kernel(num_graphs, nv, z, mask, Wc, bc, Wy, by):
    global _last_results
    G = int(num_graphs)
    NV = int(nv)
    z = np.ascontiguousarray(np.asarray(z, dtype=np.float32))
    mask = np.asarray(mask, dtype=np.float32).reshape(G, NV)
    Wc = np.asarray(Wc, dtype=np.float32)
    bc = np.asarray(bc, dtype=np.float32)
    Wy = np.asarray(Wy, dtype=np.float32)
    by = np.asarray(by, dtype=np.float32)
    H = z.shape[-1]
    zg = z.reshape(G, NV, H)

    out_full = np.zeros((G * NV, H), dtype=np.float32)

    # ---- host: active-node compaction & slot assignment ----
    act_idx = [np.nonzero(mask[g] > 0.5)[0] for g in range(G)]
    n_act = np.array([len(a) for a in act_idx])
    for g in range(G):
        if n_act[g] == 0:  # reference: 0/0 -> NaN for the whole graph
            out_full[g * NV:(g + 1) * NV, :] = np.nan

    order = np.argsort(-n_act, kind="stable")  # graphs by count, descending
    n_slots = (G + N_CORES - 1) // N_CORES
    assign = [[None] * n_slots for _ in range(N_CORES)]
    P_list = []
    for s in range(n_slots):
        ranks = order[s * N_CORES:(s + 1) * N_CORES]
        for c, g in enumerate(ranks):
            assign[c][s] = int(g)
        mx = max((int(n_act[g]) for g in ranks), default=0)
        P_list.append(max(8, (mx + 7) // 8 * 8))  # j-extent: multiple of 8
    PI_list = [max(1, max((int(n_act[g]) for g in order[s * N_CORES:(s + 1) * N_CORES]), default=1)) for s in range(n_slots)]
    offs = np.cumsum([0] + P_list[:-1]).tolist()
    NTOT = sum(P_list)
    ONB = max(128, NTOT)

    # ---- host: per-core input staging ----
    import ml_dtypes

    def _wchunks(wt):  # [256, 256] -> [128, 512] chunks (kb,ob)-major for ob0 first
        w2 = np.empty((128, 512), dtype=ml_dtypes.bfloat16)
        for ob in range(2):
            for kb in range(2):
                w2[:, (2 * ob + kb) * 128:(2 * ob + kb + 1) * 128] = (
                    wt[kb * 128:(kb + 1) * 128, ob * 128:(ob + 1) * 128]
                )
        return np.ascontiguousarray(w2)

    wcT = _wchunks(Wc.T.astype(ml_dtypes.bfloat16))  # [h_in, o] chunks
    wyT = _wchunks(Wy.T.astype(ml_dtypes.bfloat16))
    bsum = (bc + by).astype(np.float32)

    in_maps = []
    for c in range(N_CORES):
        zT_act = np.zeros((H, NTOT), dtype=ml_dtypes.bfloat16)
        madd = np.full((1, NTOT), PAD_NEG, dtype=np.float32)
        for s in range(n_slots):
            g = assign[c][s]
            if g is None:
                continue
            n = int(n_act[g])
            if n == 0:
                continue
            o = int(offs[s])
            zT_act[:, o:o + n] = zg[g][act_idx[g]].T.astype(ml_dtypes.bfloat16)
            madd[0, o:o + n] = 0.0
        zT2 = np.empty((128, 2 * NTOT), dtype=ml_dtypes.bfloat16)
        zT2[:, :NTOT] = zT_act[:128]
        zT2[:, NTOT:] = zT_act[128:]
        auxrow = np.zeros((1, NTOT + ONB + 256), dtype=ml_dtypes.bfloat16)
        auxrow[0, 0:NTOT] = madd[0].astype(ml_dtypes.bfloat16)
        auxrow[0, NTOT:NTOT + ONB] = 1.0
        auxrow[0, NTOT + ONB:NTOT + ONB + 256] = bsum.astype(ml_dtypes.bfloat16)
        in_maps.append(
            {
                "zT": np.ascontiguousarray(zT2),
                "wcT": wcT,
                "wyT": wyT,
                "aux": np.ascontiguousarray(auxrow),
            }
        )

    # ---- build + run ----
    key = (tuple(P_list), tuple(PI_list), H)
    nc = _program_cache.get(key)
    if nc is None:
        nc = _build_program(P_list, PI_list, H)
        _program_cache[key] = nc
    res = run_bass_kernel_spmd(nc, in_maps, list(range(N_CORES)))
    _last_results = res

    # ---- host: scatter back (device output is [h1, (ob, col)]-major) ----
    for c in range(N_CORES):
        oc = res.results[c]["out"].reshape(128, 2, NTOT)  # [h1, ob, col]
        for s in range(n_slots):
            g = assign[c][s]
            if g is None:
                continue
            n = int(n_act[g])
            if n == 0:
                continue
            o = int(offs[s])
            blk = oc[:, :, o:o + n]  # [128, 2, n] (unscaled sums)
            out_full[g * NV + act_idx[g], :] = (
                blk.transpose(2, 1, 0).reshape(n, H)
                * (np.float32(1.0) / np.float32(n))
            )
    return out_full


# revision 18
# speedup vs baseline: 1.1627x; 1.1627x over previous
"""Trainium2 Bass kernel for masked pairwise-sigmoid GNN message passing.

Reference computation (per graph g with nodes i,j in [0,nv)):
    c = z @ Wc.T + bc ; y = z @ Wy.T + by          # [G, nv, H]
    s[g,i,j,:] = sigmoid(c[g,i,:] + y[g,j,:] + (m_i + m_j)*L - 2L)
    out[g,i,:] = sum_j s[g,i,j,:] / sum_j m[g,j]

Exact identity: with m in {0,1}, any pair with m_i==0 or m_j==0 has mask
term <= -1e10, so sigmoid underflows to exactly 0 in fp32.  Only active
nodes (m==1) contribute; for active pairs the mask term is 0.  The host
gathers active nodes per graph, the device computes the dense active x
active interaction, and the host scatters rows back (applying the
1/n_active scale during the scatter).

Sharding: graphs sorted by active count, dealt round-robin to 8 cores in
4 slots; slot s padded to a global j-extent P_s (multiple of 4) and an
exact i-extent PI_s, so one SPMD program serves all cores.  Padding
columns get a -1e5 additive mask (sigmoid -> 0); padding rows are
discarded on scatter.

Device design (v3; every choice below is from perfetto traces on HW):
- Each dma_start costs ~640ns of issue plus ~1.3us of queue pipeline
  before data moves, so DMAs are consolidated: sync carries zT then wcT;
  gpsimd carries wyT then a 1-row aux [em | ones | bc+by].  The scalar
  queue carries NO input DMA: a dma_start on the scalar engine ahead of
  the first activation makes the act-table pass emit a second
  ACT_TABLE_LOAD (sets are chosen greedily per function), and with a
  dummy sigmoid leading the scalar stream the single sigmoid_and_others
  load (contains copy too) runs during the DMA window instead.
- bc+by and the pad mask are folded into the y-projection PSUM as two
  rank-1 matmuls (ones (x) em  and  bsum (x) ones), so all PSUM
  evacuations are plain table-free Copy ops and sigmoids need no bias.
- c' is evacuated by the DVE (idle until pass 1) as duplicated bf16
  pairs [c_i|c_i] straight from PSUM — the step-1 pairs are what enables
  the 2x_1P packed mode for the broadcast add; y' is evacuated by ACT.
- Reduction per slot: sigmoid (ACT, (224+FD)/1.2GHz, the only engine
  with a table path) then bf16 halving adds on DVE at 2 elem/cycle —
  twice when P%8==0 (the second halve's operand offset must stay
  4B-aligned) — and a final tensor_reduce (1 elem/cycle, never packs).
- GPSIMD issues DMAs only (a running gpsimd tensor op blocks
  concurrently issued 2-port DVE ops on the shared POOL SBUF port).
"""

import numpy as np

import concourse.bass as bass
import concourse.mybir as mybir
import concourse.tile as tile
from concourse import bacc
from concourse.bass_utils import run_bass_kernel_spmd

F32 = mybir.dt.float32
BF16 = mybir.dt.bfloat16
N_CORES = 8
PAD_NEG = -1.0e5  # additive mask for padding columns; sigmoid(-1e5) == 0

# test.py reads this for profiling info after a traced run
_last_results = None
_program_cache = {}


def _ap(view, free_dims):
    """AP anchored at `view`'s base with custom free dims (stride, num)."""
    return bass.AP(
        tensor=view.tensor,
        offset=view.offset,
        ap=[list(view.ap[0])] + [[int(s), int(n)] for s, n in free_dims],
    )


def _build_program(P_list, PI_list, H):
    """P_list: per-slot j-extent (mult of 4); PI_list: per-slot i-extent."""
    NTOT = sum(P_list)
    ONB = max(128, NTOT)  # ones block length (lhsT slice needs >=128)
    assert H == 256
    nc = bacc.Bacc(None, target_bir_lowering=False)

    # sync blob = [z (2*NTOT) | wc chunks (512) | wy chunks (512)]; weight
    # chunk layout per W: (kb0ob0 | kb1ob0 | kb0ob1 | kb1ob1) * 128
    BW = 2 * NTOT + 4 * H
    blob = nc.dram_tensor("blob", [128, BW], BF16, kind="ExternalInput")
    # two rows, the k=2 rank-1 operands (see below):
    #   row0: [em (NTOT)   | ones (128) | ones (128)]
    #   row1: [ones (NTOT) | bsum ob0   | bsum ob1  ]
    aux = nc.dram_tensor("aux", [2, NTOT + 256], BF16, kind="ExternalInput")
    out = nc.dram_tensor("out", [128, 2 * NTOT], F32, kind="ExternalOutput")

    AT = mybir.ActivationFunctionType
    OP = mybir.AluOpType

    offs = [0]
    for P in P_list[:-1]:
        offs.append(offs[-1] + P)

    with tile.TileContext(nc) as tc:
        with (
            tc.tile_pool(name="singles", bufs=1) as singles,
            tc.tile_pool(name="ptp", bufs=2) as ptp,
            tc.tile_pool(name="stp", bufs=2) as stp,
            tc.tile_pool(name="hvp", bufs=2) as hvp,
            tc.tile_pool(name="hqp", bufs=2) as hqp,
            tc.tile_pool(name="oup", bufs=2) as oup,
            tc.tile_pool(name="psum", bufs=1, space="PSUM") as psum,
        ):
            blob_sb = singles.tile([128, BW], BF16, tag="blob", name="blob_sb")
            z_sb = blob_sb[:, 0:2 * NTOT]
            w_sb = {
                "c": blob_sb[:, 2 * NTOT:2 * NTOT + 2 * H],
                "y": blob_sb[:, 2 * NTOT + 2 * H:2 * NTOT + 4 * H],
            }
            aux_sb = singles.tile([2, NTOT + 256], BF16, tag="aux", name="aux_sb")
            scratch = singles.tile([1, 1], F32, tag="scr", name="scratch")

            # sync: everything big in ONE dma (a queue's 2nd DMA starts its
            # data ~1.1us after the 1st ends); gpsimd: the tiny aux rows
            # (land ~1us before the blob, so the rank-1s run first);
            # scalar: NO dma (it would force a 2nd ACT_TABLE_LOAD)
            nc.sync.dma_start(out=blob_sb[:], in_=blob[:])
            nc.gpsimd.dma_start(out=aux_sb[:], in_=aux[:])
            # dummy sigmoid: pins the single table load at the top of the
            # scalar stream, overlapping the DMA window
            nc.scalar.activation(
                out=scratch[:], in_=nc.const_aps.tensor(0.0, (1, 1)),
                func=AT.Sigmoid,
            )

            # ---- projections -> PSUM (biases+mask folded in as ONE k=2
            # matmul: rows [ones;bsum] x [em;ones] add em[col] + bsum[h])
            ps_t = {}
            for ob in range(2):
                for wname in ("y", "c"):
                    ps = psum.tile(
                        [128, NTOT], F32, tag=f"ps{wname}{ob}", name=f"ps{wname}{ob}"
                    )
                    for kb in range(2):
                        o0 = (2 * ob + kb) * 128
                        nc.tensor.matmul(
                            ps[:],
                            lhsT=w_sb[wname][:, o0:o0 + 128],
                            rhs=z_sb[:, kb * NTOT:(kb + 1) * NTOT],
                            start=(kb == 0),
                            stop=(kb == 1 and wname == "c"),
                        )
                    if wname == "y":
                        nc.tensor.matmul(
                            ps[:],
                            lhsT=aux_sb[0:2, NTOT + 128 * ob:NTOT + 128 * (ob + 1)],
                            rhs=aux_sb[0:2, 0:NTOT],
                            start=False, stop=True,
                        )
                    ps_t[wname, ob] = ps

            # ---- evacuations: c' by DVE (idle) as dup pairs; y' by ACT
            c2 = singles.tile([128, 4 * NTOT], BF16, tag="c2", name="c2")
            yb = singles.tile([128, 2 * NTOT], BF16, tag="yb", name="yb")
            for ob in range(2):
                nc.vector.tensor_copy(
                    out=_ap(c2[:, 2 * ob * NTOT:2 * ob * NTOT + 2],
                            [(2, NTOT), (1, 2)]),
                    in_=_ap(ps_t["c", ob][:, 0:NTOT], [(1, NTOT), (0, 2)]),
                )
                nc.scalar.copy(
                    out=yb[:, ob * NTOT:(ob + 1) * NTOT],
                    in_=ps_t["y", ob][:, 0:NTOT],
                )

            # ---- pass 1: packed broadcast adds, SMALLEST slot first (the
            # sigmoid chain hangs off the first TT pair; ascending order
            # starts it ~1us earlier and parks the biggest tree at the end
            # where the DVE has slack)
            sorder = sorted(range(len(P_list)), key=lambda s: PI_list[s] * P_list[s])
            pts = {}
            for si in sorder:
                P, PI = P_list[si], PI_list[si]
                col = offs[si]
                pt = ptp.tile(
                    [128, 2, PI, P], BF16, tag=f"pair{si}", name=f"pair{si}"
                )
                for ob in range(2):
                    cb = 2 * ob * NTOT + 2 * col
                    in0 = _ap(c2[:, cb:cb + 2], [(2, PI), (0, P // 2), (1, 2)])
                    in1 = _ap(yb[:, ob * NTOT + col:ob * NTOT + col + P],
                              [(0, PI), (1, P)])
                    nc.vector.tensor_tensor(
                        out=pt[:, ob:ob + 1], in0=in0, in1=in1, op=OP.add
                    )
                pts[si] = pt

            # ---- pass 2: sigmoid -> halving tree -> reduce -> store
            for si in sorder:
                P, PI = P_list[si], PI_list[si]
                col = offs[si]
                pt = pts[si]
                st = stp.tile([128, 2, PI, P], BF16, tag="sig", name="sig_t")
                nc.scalar.activation(out=st[:], in_=pt[:], func=AT.Sigmoid)
                hw = P // 2
                hv = hvp.tile([128, 2, PI, hw], BF16, tag="hv", name="hv_t")
                nc.vector.tensor_tensor(
                    out=hv[:], in0=st[:, :, :, 0:hw], in1=st[:, :, :, hw:P],
                    op=OP.add,
                )
                last = hv
                w = hw
                if P % 8 == 0:  # second halve stays 4B-aligned only then
                    hq = hw // 2
                    h2 = hqp.tile([128, 2, PI, hq], BF16, tag="hq", name="hq_t")
                    nc.vector.tensor_tensor(
                        out=h2[:], in0=hv[:, :, :, 0:hq], in1=hv[:, :, :, hq:hw],
                        op=OP.add,
                    )
                    last, w = h2, hq
                red = oup.tile([128, 2, PI], F32, tag="red", name="red_t")
                nc.vector.reduce_sum(out=red[:], in_=last[:], axis=mybir.AxisListType.X)
                nc.sync.dma_start(
                    out=_ap(out[0:128, col:col + PI], [(NTOT, 2), (1, PI)]),
                    in_=red[:],
                )

    nc.finalize()
    return nc


def kernel(num_graphs, nv, z, mask, Wc, bc, Wy, by):
    global _last_results
    G = int(num_graphs)
    NV = int(nv)
    z = np.ascontiguousarray(np.asarray(z, dtype=np.float32))
    mask = np.asarray(mask, dtype=np.float32).reshape(G, NV)
    Wc = np.asarray(Wc, dtype=np.float32)
    bc = np.asarray(bc, dtype=np.float32)
    Wy = np.asarray(Wy, dtype=np.float32)
    by = np.asarray(by, dtype=np.float32)
    H = z.shape[-1]
    zg = z.reshape(G, NV, H)

    out_full = np.zeros((G * NV, H), dtype=np.float32)

    # ---- host: active-node compaction & slot assignment ----
    act_idx = [np.nonzero(mask[g] > 0.5)[0] for g in range(G)]
    n_act = np.array([len(a) for a in act_idx])
    for g in range(G):
        if n_act[g] == 0:  # reference: 0/0 -> NaN for the whole graph
            out_full[g * NV:(g + 1) * NV, :] = np.nan

    order = np.argsort(-n_act, kind="stable")  # graphs by count, descending
    n_slots = (G + N_CORES - 1) // N_CORES
    assign = [[None] * n_slots for _ in range(N_CORES)]
    P_list = []
    for s in range(n_slots):
        ranks = order[s * N_CORES:(s + 1) * N_CORES]
        for c, g in enumerate(ranks):
            assign[c][s] = int(g)
        mx = max((int(n_act[g]) for g in ranks), default=0)
        P_list.append(max(4, (mx + 3) // 4 * 4))  # j-extent: multiple of 4
    PI_list = [max(1, max((int(n_act[g]) for g in order[s * N_CORES:(s + 1) * N_CORES]), default=1)) for s in range(n_slots)]
    offs = np.cumsum([0] + P_list[:-1]).tolist()
    NTOT = sum(P_list)
    ONB = max(128, NTOT)

    # ---- host: per-core input staging ----
    import ml_dtypes

    def _wchunks(wt):  # [256, 256] -> [128, 512] chunks (kb,ob)-major for ob0 first
        w2 = np.empty((128, 512), dtype=ml_dtypes.bfloat16)
        for ob in range(2):
            for kb in range(2):
                w2[:, (2 * ob + kb) * 128:(2 * ob + kb + 1) * 128] = (
                    wt[kb * 128:(kb + 1) * 128, ob * 128:(ob + 1) * 128]
                )
        return np.ascontiguousarray(w2)

    wcT = _wchunks(Wc.T.astype(ml_dtypes.bfloat16))  # [h_in, o] chunks
    wyT = _wchunks(Wy.T.astype(ml_dtypes.bfloat16))
    bsum = (bc + by).astype(np.float32)

    in_maps = []
    for c in range(N_CORES):
        zT_act = np.zeros((H, NTOT), dtype=ml_dtypes.bfloat16)
        madd = np.full((1, NTOT), PAD_NEG, dtype=np.float32)
        for s in range(n_slots):
            g = assign[c][s]
            if g is None:
                continue
            n = int(n_act[g])
            if n == 0:
                continue
            o = int(offs[s])
            zT_act[:, o:o + n] = zg[g][act_idx[g]].T.astype(ml_dtypes.bfloat16)
            madd[0, o:o + n] = 0.0
        blob = np.empty((128, 2 * NTOT + 1024), dtype=ml_dtypes.bfloat16)
        blob[:, :NTOT] = zT_act[:128]
        blob[:, NTOT:2 * NTOT] = zT_act[128:]
        blob[:, 2 * NTOT:2 * NTOT + 512] = wcT
        blob[:, 2 * NTOT + 512:2 * NTOT + 1024] = wyT
        auxrow = np.zeros((2, NTOT + 256), dtype=ml_dtypes.bfloat16)
        auxrow[0, 0:NTOT] = madd[0].astype(ml_dtypes.bfloat16)
        auxrow[0, NTOT:NTOT + 256] = 1.0
        auxrow[1, 0:NTOT] = 1.0
        auxrow[1, NTOT:NTOT + 256] = bsum.astype(ml_dtypes.bfloat16)
        in_maps.append(
            {
                "blob": np.ascontiguousarray(blob),
                "aux": np.ascontiguousarray(auxrow),
            }
        )

    # ---- build + run ----
    key = (tuple(P_list), tuple(PI_list), H)
    nc = _program_cache.get(key)
    if nc is None:
        nc = _build_program(P_list, PI_list, H)
        _program_cache[key] = nc
    res = run_bass_kernel_spmd(nc, in_maps, list(range(N_CORES)))
    _last_results = res

    # ---- host: scatter back (device output is [h1, (ob, col)]-major) ----
    for c in range(N_CORES):
        oc = res.results[c]["out"].reshape(128, 2, NTOT)  # [h1, ob, col]
        for s in range(n_slots):
            g = assign[c][s]
            if g is None:
                continue
            n = int(n_act[g])
            if n == 0:
                continue
            o = int(offs[s])
            blk = oc[:, :, o:o + n]  # [128, 2, n] (unscaled sums)
            out_full[g * NV + act_idx[g], :] = (
                blk.transpose(2, 1, 0).reshape(n, H)
                * (np.float32(1.0) / np.float32(n))
            )
    return out_full
